# revision 16
# baseline (speedup 1.0000x reference)
"""Trainium2 Bass kernel for nn_ContrastiveLoss (segment_reduce).

The loss depends only on the masked segment means, and a comb row
(mA_i & mB_i) is usually empty or touches a tiny fraction of the image.
Host-side *mask-only* analysis finds
  - the valid objects (nonzero comb rows; the rest enter the loss only as
    exp(0)=1 constants inside the logsumexp, folded into the Ln bias), and
  - the BS-px pixel blocks each valid comb row touches; P/BS blocks pack
    into one 128-partition contraction tile.

Fast path (sparse masks, the expected regime): ONE launch on one core —
the few selected feature/comb tiles stream in fp8, PE accumulates the
segment sums psT[ch, v] (output free size = Vp), and the loss chain
(norms via exp(-0.5*ln(nsq)), Vp x Vp logits, lse - diag, masked mean)
runs on the same core. No cross-core combine, no second-launch DMA
latency, full f32 partials.

Fallback (dense masks, > 128 tiles): two launches — 8 cores compute
bf16 partial sums data-parallel over tiles, core 0 combines.

fp8 features keep rel err ~4e-3 (tolerance 2e-2); the /cnt of the
reference cancels inside l2norm; pad == (sk[:,0] != 0) is reproduced
exactly on-chip. All Act ops (Ln, Exp) live in one table set, warmed
during the input DMA. PSUM accumulation groups never interleave within
a bank (that corrupts results).
"""

import numpy as np
from contextlib import ExitStack

import concourse.bass as bass
import concourse.tile as tile
from concourse import bacc, mybir
from concourse.bass_utils import run_bass_kernel_spmd

# Problem constants (hardcoded per task spec)
B, M, C, H, W = 4, 50, 256, 100, 352
HW = H * W                  # 35200
N = B * M                   # 200
TAU = 0.07
P = 128                     # partitions / contraction tile
BS = 4                      # selection block: 4 consecutive pixels
BPT = P // BS               # blocks packed per tile
NB_FULL = HW // BS

F32 = mybir.dt.float32
BF16 = mybir.dt.bfloat16
FP8 = mybir.dt.float8e4
NP_FP8 = mybir.dt.np(FP8)

# Force exp/ln to resolve to the combined "natural_log_exp_and_others" table
# set (index 6): empty the earlier sets we never want so first-match lands on
# sqrt_and_others (3) for sqrt/copy and natural_log_exp_and_others (6) for
# exp+ln. Indices are preserved so act_func_set_id stays aligned.
import concourse.bacc as _bacc_mod
import concourse.hw_specs as _hw_specs
_orig_get_tables = _hw_specs.get_activation_tables


def _patched_get_tables(module_arch):
    tables = dict(_orig_get_tables(module_arch))
    for i, k in enumerate(tables):
        if i in (0, 1, 2, 4, 5):
            tables[k] = set()
    return tables


_bacc_mod.get_activation_tables = _patched_get_tables

_cache = {}


def _emit_consts(nc, sb, Vp, blocks):
    """Constant tiles + activation warm-up (all overlap the input DMA)."""
    ones = sb.tile([P, 1], F32, name="ones")
    nc.gpsimd.memset(ones[:], 1.0)
    onesr = sb.tile([1, P], F32, name="onesr")
    nc.gpsimd.memset(onesr[:], 1.0)
    itaur = sb.tile([1, P], F32, name="itaur")
    nc.gpsimd.memset(itaur[:], 1.0 / TAU)
    idbs = []
    for i0, rows in blocks:
        idb = sb.tile([P, Vp], F32, name=f"idb{i0}")
        nc.gpsimd.memset(idb[:], 0.0)
        nc.gpsimd.affine_select(
            out=idb[:], in_=idb[:],
            compare_op=mybir.AluOpType.not_equal, fill=1.0, base=i0,
            pattern=[[-1, Vp]], channel_multiplier=1)
        idbs.append(idb)
    ceps = sb.tile([1, 1], F32, name="ceps")
    nc.gpsimd.memset(ceps[:], 1e-30)
    czc = sb.tile([P, 1], F32, name="czc")
    nc.gpsimd.memset(czc[:], float(N - Vp))
    # warm the exp/ln table set; every Act op here lives in that single set
    w1 = sb.tile([1, 1], F32, name="w1")
    nc.scalar.activation(w1, onesr[0:1, 0:1],
                         mybir.ActivationFunctionType.Exp)
    return dict(ones=ones, onesr=onesr, itaur=itaur, idbs=idbs,
                ceps=ceps, czc=czc)


def _emit_loss(nc, sb, psum, ST, Vp, cst, out):
    """Loss from ST (128ch, nm, cb, Vp) f32 segment sums; DMAs to `out`."""
    blocks = [(i0, min(P, Vp - i0)) for i0 in range(0, Vp, P)]
    one1 = cst["onesr"][0:1, 0:1]

    # raw logit matmuls only need ST: issue them first so the in-order
    # PE queue runs them during the norm chain
    ps_Ls = []
    for bi, (i0, rows) in enumerate(blocks):
        ps_L = psum.tile([P, Vp], F32, name=f"ps_L{bi}", tag="ps")
        for cb in range(2):
            nc.tensor.matmul(ps_L[:rows], ST[:, 1, cb, i0:i0 + rows],
                             ST[:, 0, cb, :], start=(cb == 0),
                             stop=(cb == 1))
        ps_Ls.append(ps_L)

    # row norms^2, then 1/norm = exp(-0.5 * ln(nsq + eps))
    sq = sb.tile([P, 2, 2, Vp], F32, name="sq")
    nc.vector.tensor_mul(sq, ST, ST)
    ns_ps = psum.tile([1, 2, Vp], F32, name="ns_ps", tag="ps")
    for cb in range(2):
        nc.tensor.matmul(ns_ps, cst["ones"], sq[:, :, cb, :],
                         start=(cb == 0), stop=(cb == 1))
    lnn = sb.tile([1, 2, Vp], F32, name="lnn")
    nc.scalar.activation(lnn, ns_ps, mybir.ActivationFunctionType.Ln,
                         bias=cst["ceps"])
    inv = sb.tile([1, 2, Vp], F32, name="inv")
    nc.scalar.activation(inv, lnn, mybir.ActivationFunctionType.Exp,
                         scale=-0.5)

    padrow = sb.tile([1, Vp], F32, name="padrow")
    nc.vector.tensor_scalar(padrow, ST[0:1, 1, 0, :], 0.0, None,
                            op0=mybir.AluOpType.not_equal)

    nd = psum.tile([1, 2, len(blocks)], F32, name="nd", tag="ps")
    for bi, (i0, rows) in enumerate(blocks):
        # a_i = inv_k[i0+i] / tau (the 1/tau rides the psum->sbuf copy;
        # PE transpose is pure data movement and ignores identity values)
        a_ps = psum.tile([P, 1], F32, name="a_ps", tag="ps")
        nc.tensor.transpose(a_ps[:rows], inv[:, 1, i0:i0 + rows], one1)
        p_ps = psum.tile([P, 1], F32, name="p_ps", tag="ps")
        nc.tensor.transpose(p_ps[:rows], padrow[:, i0:i0 + rows], one1)
        acol = sb.tile([P, 1], F32, name="acol")
        nc.vector.tensor_scalar_mul(acol[:rows], a_ps[:rows], 1.0 / TAU)
        pcol = sb.tile([P, 1], F32, name="pcol")
        nc.vector.tensor_copy(pcol[:rows], p_ps[:rows])
        # per-col scale Bb[i, j] = inv_q[j] on the (otherwise idle) Pool
        Bb = sb.tile([P, Vp], F32, name="Bb")
        nc.gpsimd.partition_broadcast(Bb[:rows], inv[:, 0, :])
        lg = sb.tile([P, Vp], F32, name="lg")
        nc.vector.scalar_tensor_tensor(lg[:rows], ps_Ls[bi][:rows],
                                       acol[:rows], Bb[:rows],
                                       op0=mybir.AluOpType.mult,
                                       op1=mybir.AluOpType.mult)
        # lse without max subtraction (|logits| <= ~14.3 is exp-safe);
        # Ln bias folds the N - Vp all-zero columns (each exp(0) = 1)
        es = sb.tile([P, Vp], F32, name="es")
        ssum = sb.tile([P, 1], F32, name="ssum")
        nc.scalar.activation(es[:rows], lg[:rows],
                             mybir.ActivationFunctionType.Exp,
                             accum_out=ssum[:rows])
        lse = sb.tile([P, 1], F32, name="lse")
        nc.scalar.activation(lse[:rows], ssum[:rows],
                             mybir.ActivationFunctionType.Ln,
                             bias=cst["czc"][:rows])
        # diag via masked row-sum
        dsel = sb.tile([P, Vp], F32, name="dsel")
        nc.vector.tensor_mul(dsel[:rows], lg[:rows], cst["idbs"][bi][0:rows, :])
        dcol = sb.tile([P, 1], F32, name="dcol")
        nc.vector.tensor_reduce(dcol[:rows], dsel[:rows],
                                axis=mybir.AxisListType.X,
                                op=mybir.AluOpType.add)
        ce = sb.tile([P, 1], F32, name="ce")
        nc.vector.scalar_tensor_tensor(ce[:rows], lse[:rows], dcol[:rows],
                                       pcol[:rows],
                                       op0=mybir.AluOpType.subtract,
                                       op1=mybir.AluOpType.mult)
        nc.tensor.matmul(nd[:, 0, bi:bi + 1], cst["ones"][0:rows], ce[:rows],
                         start=True, stop=True)
        nc.tensor.matmul(nd[:, 1, bi:bi + 1], cst["ones"][0:rows],
                         pcol[:rows], start=True, stop=True)

    num = sb.tile([1, 2], F32, name="num")
    if len(blocks) == 1:
        nc.vector.tensor_copy(num[:, 0:1], nd[:, 0, :])
        nc.vector.tensor_copy(num[:, 1:2], nd[:, 1, :])
    else:
        nc.vector.tensor_reduce(num, nd, axis=mybir.AxisListType.X,
                                op=mybir.AluOpType.add)
    den = sb.tile([1, 1], F32, name="den")
    nc.vector.tensor_scalar_max(den, num[:, 1:2], 1.0)
    rden = sb.tile([1, 1], F32, name="rden")
    nc.vector.reciprocal(rden, den)
    res = sb.tile([1, 1], F32, name="res")
    nc.vector.tensor_mul(res, num[:, 0:1], rden)
    nc.sync.dma_start(out=out[:], in_=res)


def _emit_segsum_matmuls(nc, fin_a, fin_b, psum, T, Vp):
    """Accumulate psT[ch, nm, cb, v] over T tiles. fin_a = [q | comb],
    fin_b = [k]. Groups run back-to-back (interleaved open groups within
    a PSUM bank corrupt results)."""
    COFF = T * 256
    ps = psum.tile([P, 2, 2, Vp], F32, name="ps")
    for nm, src in ((0, fin_a), (1, fin_b)):
        for cb in range(2):
            for t in range(T):
                lo = t * 256 + cb * P
                cmb = fin_a[:, COFF + t * Vp: COFF + (t + 1) * Vp]
                nc.tensor.matmul(ps[:, nm, cb, :], src[:, lo:lo + P],
                                 cmb, start=(t == 0), stop=(t == T - 1))
    return ps


def _build_single(T, Vp):
    """One launch, one core: segment sums + loss."""
    FS = 2 * T * 256 + T * Vp
    KOFF = T * 256 + T * Vp
    nc = bacc.Bacc(None, target_bir_lowering=False, debug=False)
    blocks = [(i0, min(P, Vp - i0)) for i0 in range(0, Vp, P)]
    with tile.TileContext(nc) as tc, ExitStack() as ctx:
        dram = ctx.enter_context(tc.tile_pool(name="dram", bufs=1, space="DRAM"))
        fin = dram.tile([P, FS], FP8, kind="ExternalInput", name="fin",
                        uniquify=False)
        out = dram.tile([1, 1], F32, kind="ExternalOutput", name="loss",
                        uniquify=False)
        sb = ctx.enter_context(tc.tile_pool(name="sb", bufs=1))
        psum = ctx.enter_context(tc.tile_pool(name="psum", bufs=3, space="PSUM"))

        cst = _emit_consts(nc, sb, Vp, blocks)

        fin_a = sb.tile([P, KOFF], FP8, name="fin_a")
        fin_b = sb.tile([P, T * 256], FP8, name="fin_b")
        nc.sync.dma_start(out=fin_a, in_=fin[:, :KOFF])
        nc.sync.dma_start(out=fin_b, in_=fin[:, KOFF:])

        ps = _emit_segsum_matmuls(nc, fin_a, fin_b, psum, T, Vp)
        ST = sb.tile([P, 2, 2, Vp], F32, name="ST")
        nc.vector.tensor_copy(ST, ps)
        _emit_loss(nc, sb, psum, ST, Vp, cst, out)
    nc.compile()
    return nc


def _build_phase1(T, Vp):
    """Fallback launch 1 (8 cores): bf16 partial segment sums."""
    FS = 2 * T * 256 + T * Vp
    KOFF = T * 256 + T * Vp
    nc = bacc.Bacc(None, target_bir_lowering=False, debug=False)
    with tile.TileContext(nc) as tc, ExitStack() as ctx:
        dram = ctx.enter_context(tc.tile_pool(name="dram", bufs=1, space="DRAM"))
        fin = dram.tile([P, FS], FP8, kind="ExternalInput", name="fin",
                        uniquify=False)
        pout = dram.tile([P, 4 * Vp], BF16, kind="ExternalOutput", name="pout",
                         uniquify=False)
        sb = ctx.enter_context(tc.tile_pool(name="sb", bufs=1))
        psum = ctx.enter_context(tc.tile_pool(name="psum", bufs=1, space="PSUM"))

        fin_a = sb.tile([P, KOFF], FP8, name="fin_a")
        fin_b = sb.tile([P, T * 256], FP8, name="fin_b")
        nc.sync.dma_start(out=fin_a, in_=fin[:, :KOFF])
        nc.sync.dma_start(out=fin_b, in_=fin[:, KOFF:])

        ps = _emit_segsum_matmuls(nc, fin_a, fin_b, psum, T, Vp)
        o = sb.tile([P, 2, 2, Vp], BF16, name="o")
        nc.vector.tensor_copy(o, ps)
        nc.sync.dma_start(out=pout[:], in_=o)
    nc.compile()
    return nc


def _build_phase2(Vp):
    """Fallback launch 2 (1 core): combine 8 cores' partials into the loss."""
    nc = bacc.Bacc(None, target_bir_lowering=False, debug=False)
    blocks = [(i0, min(P, Vp - i0)) for i0 in range(0, Vp, P)]
    with tile.TileContext(nc) as tc, ExitStack() as ctx:
        dram = ctx.enter_context(tc.tile_pool(name="dram", bufs=1, space="DRAM"))
        pin = dram.tile([P, 2, 2, Vp, 8], BF16, kind="ExternalInput",
                        name="pin", uniquify=False)
        out = dram.tile([1, 1], F32, kind="ExternalOutput", name="loss",
                        uniquify=False)
        sb = ctx.enter_context(tc.tile_pool(name="sb", bufs=1))
        psum = ctx.enter_context(tc.tile_pool(name="psum", bufs=3, space="PSUM"))

        cst = _emit_consts(nc, sb, Vp, blocks)
        raw = sb.tile([P, 2, 2, Vp, 8], BF16, name="raw")
        nc.sync.dma_start(out=raw, in_=pin[:])
        ST = sb.tile([P, 2, 2, Vp], F32, name="ST")
        nc.vector.tensor_reduce(ST, raw, axis=mybir.AxisListType.X,
                                op=mybir.AluOpType.add)
        _emit_loss(nc, sb, psum, ST, Vp, cst, out)
    nc.compile()
    return nc


def _analyze(mask):
    """Mask-only analysis: valid objects and their (batch, block) work items."""
    mask2 = mask.reshape(B, M, HW)
    mask_flat = mask2.reshape(N, HW)
    ii = np.arange(N)
    comb = mask_flat & mask2[ii % B, ii // B]      # (N, HW)
    vidx = np.nonzero(comb.any(axis=1))[0]
    V = len(vidx)
    if V == 0:
        return comb, vidx, 0, 0, []
    Vp = min(N, max(8, -(-V // 8) * 8))
    combT = comb[vidx].reshape(V, NB_FULL, BS).any(axis=2)  # (V, NB_FULL)
    items = []
    for r in range(B):
        sel = (vidx % B) == r
        if sel.any():
            for t in np.nonzero(combT[sel].any(axis=0))[0]:
                items.append((r, int(t)))
    return comb, vidx, V, Vp, items


def _host_prep(features_q, features_k, pos_region_ranges):
    """Pack mask-selected 16-px feature/comb blocks into contraction tiles."""
    fq = np.asarray(features_q, dtype=np.float32).reshape(B, C, HW)
    fk = np.asarray(features_k, dtype=np.float32).reshape(B, C, HW)
    mask = np.asarray(pos_region_ranges).astype(bool)

    comb, vidx, V, Vp, items = _analyze(mask)
    if V == 0:
        return None
    total_tiles = max(1, -(-len(items) // BPT))
    single = total_tiles <= 128
    ncores = 1 if single else 8
    tiles = [items[j * BPT:(j + 1) * BPT] for j in range(total_tiles)]
    percore = [tiles[c::ncores] for c in range(ncores)]
    T = max(len(pc) for pc in percore)
    FS = 2 * T * 256 + T * Vp

    combV = comb[vidx]                              # (V, HW)
    rsel = [(vidx % B) == r for r in range(B)]
    in_maps = []
    for c in range(ncores):
        fused = np.zeros((P, FS), NP_FP8)
        fq_v = fused[:, :T * 256].reshape(P, T, 256)
        cb_v = fused[:, T * 256:T * 256 + T * Vp].reshape(P, T, Vp)
        fk_v = fused[:, T * 256 + T * Vp:].reshape(P, T, 256)
        for j, blks in enumerate(percore[c]):
            for bi, (r, blk) in enumerate(blks):
                rows = slice(bi * BS, (bi + 1) * BS)
                sl = slice(blk * BS, (blk + 1) * BS)
                fq_v[rows, j, :] = fq[r][:, sl].T.astype(NP_FP8)
                fk_v[rows, j, :] = fk[r][:, sl].T.astype(NP_FP8)
                ct = np.where(rsel[r][:, None], combV[:, sl], False)
                cb_v[rows, j, :V] = ct.T.astype(NP_FP8)
        in_maps.append({"fin": fused})
    return in_maps, V, Vp, T, single


def kernel(features_q, features_k, pos_region_ranges):
    prep = _host_prep(features_q, features_k, pos_region_ranges)
    if prep is None:
        return np.float32(0.0)
    in_maps, V, Vp, T, single = prep

    if single:
        key = ("s", T, Vp)
        if key not in _cache:
            _cache[key] = _build_single(T, Vp)
        nc = _cache[key]
        _cache["single"] = nc
        _cache.pop("p1", None); _cache.pop("p2", None)
        r = run_bass_kernel_spmd(nc, in_maps, core_ids=[0])
        return np.float32(r.results[0]["loss"][0, 0])

    key = ("d", T, Vp)
    if key not in _cache:
        _cache[key] = (_build_phase1(T, Vp), _build_phase2(Vp))
    nc1, nc2 = _cache[key]
    _cache["p1"], _cache["p2"] = nc1, nc2
    _cache.pop("single", None)
    r1 = run_bass_kernel_spmd(nc1, in_maps, core_ids=list(range(8)))
    pin = np.stack([np.asarray(r1.results[s]["pout"]) for s in range(8)],
                   axis=-1).reshape(P, 2, 2, Vp, 8)
    r2 = run_bass_kernel_spmd(nc2, [{"pin": pin}], core_ids=[0])
    return np.float32(r2.results[0]["loss"][0, 0])
